# revision 1
# baseline (speedup 1.0000x reference)
"""BitLinear (per-token int8 activation quant + ternary weight quant + matmul)
as a Bass/Tile kernel on 8 Trainium2 NeuronCores.

Strategy (data-parallel tokens, zero collectives):
  - x [4,2048,4096] -> [8192,4096]; each core quantizes and matmuls its own
    1024-token slab against the FULL weight; outputs concatenate on tokens.
  - Every core computes mean(|W|) itself (one 67MB streaming pass that
    overlaps activation quant), then ternarizes W slab-by-slab just in time
    for the matmul, one out_feature slab (512 cols) ahead of the PE.
    No collectives => no NRT entry barrier, no AllGather serialization.
  - q = rint(x*s) (s = 127/max(|x|) per token) and tw in {-1,0,1} are exact
    in bf16 => the bf16 matmul with fp32 PSUM accumulation is EXACT integer
    arithmetic; per-token dequant scales applied on the PSUM->SBUF copy.
  - Operand transposes (contraction on partitions) via DMA xbar SBUF->SBUF.
  - The last 4 row-blocks of the mean pass are slab 0's, kept resident so
    slab 0 ternarizes without re-reads and the matmul starts immediately
    after the mean finishes.
"""
import numpy as np
from contextlib import ExitStack

N_CORES = 8
B, S, D_IN, D_OUT = 4, 2048, 4096, 4096
TOK = B * S                  # 8192
TOK_PC = TOK // N_CORES      # 1024 tokens per core
N_TOK_TILES = TOK_PC // 128  # 8
N_K = D_IN // 128            # 32 contraction tiles
OF_CHUNK = 512
N_SLAB = D_OUT // OF_CHUNK   # 8
NWB = D_OUT // 128           # 32 weight row-blocks
EPS = 1e-5
MAGIC = float(np.float32(1.5 * 2 ** 23))   # fp32 round-to-nearest-even trick
MEAN_SCALE = float(np.float32(1.0 / (D_IN * D_OUT)))  # 2^-24, exact

_CACHE = {}


def _build_module():
    import concourse.bacc as bacc
    import concourse.tile as tile
    import concourse.mybir as mybir
    import concourse.bass_isa as bass_isa

    dt = mybir.dt
    AF = mybir.ActivationFunctionType
    AL = mybir.AluOpType
    AX = mybir.AxisListType

    nc = bacc.Bacc(
        "TRN2", target_bir_lowering=False, debug=False, num_devices=N_CORES
    )
    xs = nc.dram_tensor("xs", [TOK_PC, D_IN], dt.float32, kind="ExternalInput").ap()
    wf = nc.dram_tensor("wf", [D_OUT, D_IN], dt.float32, kind="ExternalInput").ap()
    out = nc.dram_tensor("out", [TOK_PC, D_OUT], dt.float32, kind="ExternalOutput").ap()

    with tile.TileContext(nc) as tc, ExitStack() as ctx:
        stats = ctx.enter_context(tc.tile_pool(name="stats", bufs=1))
        qT_pool = ctx.enter_context(tc.tile_pool(name="qT", bufs=N_TOK_TILES))
        big = ctx.enter_context(tc.tile_pool(name="big", bufs=2))
        qb_pool = ctx.enter_context(tc.tile_pool(name="qbp", bufs=2))
        twTp = ctx.enter_context(tc.tile_pool(name="twT", bufs=2))
        op = ctx.enter_context(tc.tile_pool(name="op", bufs=2))
        pp = ctx.enter_context(tc.tile_pool(name="pp", bufs=6, space="PSUM"))

        amc = stats.tile([128, N_TOK_TILES], dt.float32, tag="amc")
        s_all = stats.tile([128, N_TOK_TILES], dt.float32, tag="s_all")
        dq = stats.tile([128, N_TOK_TILES], dt.float32, tag="dq")
        wme = stats.tile([128, 1], dt.float32, tag="wme")
        swt = stats.tile([128, 1], dt.float32, tag="swt")
        wp = stats.tile([128, NWB], dt.float32, tag="wp")
        w32 = stats.tile([128, 32], dt.float32, tag="w32")
        z32 = stats.tile([128, 32], dt.float32, tag="z32")
        z32t = stats.tile([128, 32], dt.float32, tag="z32t")
        zr = stats.tile([128, 1], dt.float32, tag="zr")
        wsum_sb = stats.tile([128, 1], dt.float32, tag="wsum_sb")
        gtot = stats.tile([128, 1], dt.float32, tag="gtot")

        # ---- x-quant: own tokens -> resident qT tiles (half tiles) ----
        HD = D_IN // 2
        HK = N_K // 2
        qT_tiles = []
        with nc.named_scope("xquant"), tc.tile_pool(name="xq", bufs=3) as xq:
            for t in range(N_TOK_TILES):
                qT_t = qT_pool.tile(
                    [128, N_K, 128], dt.bfloat16, tag="qT", name=f"qT{t}"
                )
                xh = []
                for h in range(2):
                    xth = xq.tile([128, HD], dt.float32, tag="xq", name=f"xt{t}_{h}")
                    nc.sync.dma_start(
                        xth[:], xs[t * 128:(t + 1) * 128, h * HD:(h + 1) * HD]
                    )
                    nc.vector.tensor_reduce(
                        amc[:, t:t + 1] if h == 0 else wsum_sb[:],
                        xth[:], axis=AX.X, op=AL.max, apply_absolute_value=True,
                    )
                    xh.append(xth)
                # amax = max(half0, half1); then clip, s = 127/amax_c
                nc.vector.tensor_tensor(
                    amc[:, t:t + 1], amc[:, t:t + 1], wsum_sb[:], op=AL.max
                )
                nc.vector.tensor_scalar(
                    amc[:, t:t + 1], amc[:, t:t + 1], EPS, None, op0=AL.max
                )
                nc.vector.reciprocal(s_all[:, t:t + 1], amc[:, t:t + 1])
                nc.vector.tensor_scalar(
                    s_all[:, t:t + 1], s_all[:, t:t + 1], 127.0, None, op0=AL.mult
                )
                for h in range(2):
                    nc.scalar.activation(
                        xh[h][:], xh[h][:], AF.Copy, scale=s_all[:, t:t + 1]
                    )
                    qbh = qb_pool.tile(
                        [128, HD], dt.bfloat16, tag="qb", name=f"qb{t}_{h}"
                    )
                    nc.vector.tensor_scalar(
                        qbh[:], xh[h][:], MAGIC, MAGIC, op0=AL.add, op1=AL.subtract
                    )
                    nc.sync.dma_start(
                        qT_t[:, h * HK:(h + 1) * HK, :], qbh[:], transpose=True
                    )
                qT_tiles.append(qT_t)

        # ---- |W| mean pass: stream full W on the scalar queue ----
        # order: blocks of slabs 1..7 first, then slab 0's blocks (kept hot)
        mean_order = list(range(4, NWB)) + [0, 1, 2, 3]
        kept = {}
        with nc.named_scope("wmean"):
            for idx, j in enumerate(mean_order):
                wt = big.tile([128, D_IN], dt.float32, tag="big", name=f"wm{j}")
                eng = nc.scalar if idx % 2 == 0 else nc.sync
                eng.dma_start(wt[:], wf[j * 128:(j + 1) * 128, :])
                nc.vector.tensor_reduce(
                    w32[:],
                    wt[:].rearrange("p (a b) -> p a b", b=128),
                    axis=AX.X, op=AL.add, apply_absolute_value=True,
                )
                nc.vector.tensor_reduce(
                    wp[:, j:j + 1], w32[:], axis=AX.X, op=AL.add
                )

            nc.vector.tensor_reduce(wsum_sb[:], wp[:], axis=AX.X, op=AL.add)
            # exact-ish partition reduce: 32x32 transpose -> rows 0/32/64/96
            # hold 32-sums, then partition_all_reduce adds only 4 nonzeros
            nc.vector.memset(z32[:], 0.0)
            nc.vector.tensor_copy(z32[:, 0:1], wsum_sb[:])
            nc.vector.transpose(z32t[:], z32[:])
            nc.vector.tensor_reduce(zr[:], z32t[:], axis=AX.X, op=AL.add)
            nc.gpsimd.partition_all_reduce(
                gtot[:], zr[:], channels=128, reduce_op=bass_isa.ReduceOp.add
            )
            nc.vector.tensor_scalar(
                wme[:], gtot[:], MEAN_SCALE, EPS, op0=AL.mult, op1=AL.max
            )
            nc.vector.reciprocal(swt[:], wme[:])
            for t in range(N_TOK_TILES):
                nc.vector.tensor_scalar(
                    dq[:, t:t + 1], amc[:, t:t + 1], wme[:, 0:1],
                    float(np.float32(1.0 / 127.0)), op0=AL.mult, op1=AL.mult,
                )

        # ---- per-slab: ternarize+transpose one slab ahead, then matmul ----
        def stage_tern(c):
            twT_c = twTp.tile(
                [128, N_K, OF_CHUNK], dt.bfloat16, tag="twT", name=f"twT{c}"
            )
            for j in range(4):
                blk = 4 * c + j
                wt = big.tile(
                    [128, D_IN], dt.float32, tag="big", name=f"wt{blk}"
                )
                nc.scalar.dma_start(wt[:], wf[blk * 128:(blk + 1) * 128, :])
                nc.scalar.activation(wt[:], wt[:], AF.Copy, scale=swt[:, 0:1])
                twr = qb_pool.tile([128, D_IN], dt.bfloat16, tag="qb", name=f"twr{blk}")
                nc.vector.tensor_scalar(
                    twr[:], wt[:], MAGIC, MAGIC, op0=AL.add, op1=AL.subtract
                )
                twc = qb_pool.tile([128, D_IN], dt.bfloat16, tag="qb", name=f"twc{blk}")
                nc.vector.tensor_scalar(
                    twc[:], twr[:], 1.0, -1.0, op0=AL.min, op1=AL.max
                )
                nc.sync.dma_start(
                    twT_c[:, :, j * 128:(j + 1) * 128], twc[:], transpose=True
                )
            return twT_c

        def stage_mm(c, twT_c):
            for t in range(N_TOK_TILES):
                ps = pp.tile([128, OF_CHUNK], dt.float32, tag="ps", name=f"ps{c}_{t}")
                for k in range(N_K):
                    nc.tensor.matmul(
                        ps[:], qT_tiles[t][:, k, :], twT_c[:, k, :],
                        start=(k == 0), stop=(k == N_K - 1),
                    )
                ot = op.tile([128, OF_CHUNK], dt.float32, tag="ot", name=f"ot{c}_{t}")
                nc.vector.tensor_scalar(
                    ot[:], ps[:], dq[:, t:t + 1], None, op0=AL.mult
                )
                nc.gpsimd.dma_start(
                    out[t * 128:(t + 1) * 128, c * OF_CHUNK:(c + 1) * OF_CHUNK],
                    ot[:],
                )

        with nc.named_scope("mm"):
            twT_cur = stage_tern(0)
            for c in range(N_SLAB):
                twT_next = stage_tern(c + 1) if c + 1 < N_SLAB else None
                stage_mm(c, twT_cur)
                twT_cur = twT_next

    nc.compile()
    return nc


def _get_module():
    if "nc" not in _CACHE:
        _CACHE["nc"] = _build_module()
    return _CACHE["nc"]


def _make_in_maps(x2, w2):
    return [
        {
            "xs": x2[i * TOK_PC:(i + 1) * TOK_PC],
            "wf": w2,
        }
        for i in range(N_CORES)
    ]


def kernel(x: np.ndarray, weight: np.ndarray) -> np.ndarray:
    from concourse.bass_utils import run_bass_kernel_spmd

    x = np.asarray(x, dtype=np.float32)
    weight = np.asarray(weight, dtype=np.float32)
    x2 = np.ascontiguousarray(x.reshape(TOK, D_IN))
    w2 = np.ascontiguousarray(weight)

    in_maps = _make_in_maps(x2, w2)
    nc = _get_module()
    res = run_bass_kernel_spmd(nc, in_maps, list(range(N_CORES)))
    out = np.concatenate([res.results[i]["out"] for i in range(N_CORES)], axis=0)
    return out.reshape(B, S, D_OUT)



# revision 3
# speedup vs baseline: 1.0504x; 1.0504x over previous
"""BitLinear (per-token int8 activation quant + ternary weight quant + matmul)
as a Bass/Tile kernel on 8 Trainium2 NeuronCores.

Strategy (data-parallel tokens + tensor-parallel weight-mean):
  - x [4,2048,4096] -> [8192,4096]; each core quantizes and matmuls its own
    1024-token slab against the FULL weight; outputs concatenate on tokens.
  - mean(|W|): each core reduces only its OWN 1/8 shard of W (extra input
    "ws", 8.4MB) on GpSimd, then a 512B AllReduce combines the partials.
    This removes the 67MB serial full-W mean pass from the critical path
    (the old version spent ~370us there before the first matmul).
  - q = rint(x*s) (s = 127/max(|x|) per token) and tw in {-1,0,1} are exact
    in bf16 => the bf16 matmul with fp32 PSUM accumulation is EXACT integer
    arithmetic; per-token dequant scales applied on the PSUM->SBUF copy.
  - Engine/queue split: x loads on sync-HWDGE, qT transposes on scalar-HWDGE,
    W slab streams on gpsimd-SWDGE, twT transposes on sync-HWDGE, x-quant
    arithmetic entirely on Vector, tern scale + dequant on Scalar.
  - Ternarize slab c+1 one slab ahead of the matmul on slab c; transpose
    destinations are contiguous ([128,4,32,128] layout) for xbar speed.
"""
import numpy as np
from contextlib import ExitStack

N_CORES = 8
B, S, D_IN, D_OUT = 4, 2048, 4096, 4096
TOK = B * S                  # 8192
TOK_PC = TOK // N_CORES      # 1024 tokens per core
N_TOK_TILES = TOK_PC // 128  # 8
N_K = D_IN // 128            # 32 contraction tiles
OF_CHUNK = 512
N_SLAB = D_OUT // OF_CHUNK   # 8
SHARD_ROWS = D_OUT // N_CORES  # 512 weight rows per core for the mean
EPS = 1e-5
MAGIC = float(np.float32(1.5 * 2 ** 23))   # fp32 round-to-nearest-even trick
MEAN_SCALE = float(np.float32(1.0 / (D_IN * D_OUT)))  # 2^-24, exact
INV127 = float(np.float32(1.0 / 127.0))

_CACHE = {}


def _build_module():
    import concourse.bacc as bacc
    import concourse.tile as tile
    import concourse.mybir as mybir
    import concourse.bass_isa as bass_isa

    dt = mybir.dt
    AF = mybir.ActivationFunctionType
    AL = mybir.AluOpType
    AX = mybir.AxisListType

    nc = bacc.Bacc(
        "TRN2", target_bir_lowering=False, debug=False, num_devices=N_CORES
    )
    xs = nc.dram_tensor("xs", [TOK_PC, D_IN], dt.float32, kind="ExternalInput").ap()
    wf = nc.dram_tensor("wf", [D_OUT, D_IN], dt.float32, kind="ExternalInput").ap()
    ws = nc.dram_tensor("ws", [SHARD_ROWS, D_IN], dt.float32, kind="ExternalInput").ap()
    out = nc.dram_tensor("out", [TOK_PC, D_OUT], dt.float32, kind="ExternalOutput").ap()

    with tile.TileContext(nc) as tc, ExitStack() as ctx:
        stats = ctx.enter_context(tc.tile_pool(name="stats", bufs=1))
        qT_pool = ctx.enter_context(tc.tile_pool(name="qT", bufs=N_TOK_TILES))
        big = ctx.enter_context(tc.tile_pool(name="big", bufs=2))
        qb_pool = ctx.enter_context(tc.tile_pool(name="qbp", bufs=2))
        twTp = ctx.enter_context(tc.tile_pool(name="twT", bufs=2))
        op = ctx.enter_context(tc.tile_pool(name="op", bufs=2))
        pp = ctx.enter_context(tc.tile_pool(name="pp", bufs=6, space="PSUM"))
        dram = ctx.enter_context(tc.tile_pool(name="dram", bufs=2, space="DRAM"))

        amc = stats.tile([128, N_TOK_TILES], dt.float32, tag="amc")
        am2 = stats.tile([128, N_TOK_TILES], dt.float32, tag="am2")
        sca = stats.tile([128, N_TOK_TILES], dt.float32, tag="sca")
        dq = stats.tile([128, N_TOK_TILES], dt.float32, tag="dq")
        wme = stats.tile([128, 1], dt.float32, tag="wme")
        swt = stats.tile([128, 1], dt.float32, tag="swt")
        wp = stats.tile([128, 4], dt.float32, tag="wp")
        w32 = stats.tile([128, 32], dt.float32, tag="w32")
        z32 = stats.tile([128, 32], dt.float32, tag="z32")
        z32t = stats.tile([128, 32], dt.float32, tag="z32t")
        zr = stats.tile([128, 1], dt.float32, tag="zr")
        wsum = stats.tile([128, 1], dt.float32, tag="wsum")
        gtot = stats.tile([128, 1], dt.float32, tag="gtot")
        gl = stats.tile([128, 1], dt.float32, tag="gl")

        arin = dram.tile([128, 1], dt.float32, tag="arin")
        arout = dram.tile([128, 1], dt.float32, tag="arout")

        # ---- |W| mean: reduce own 1/8 shard on gpsimd, AllReduce partials ----
        with nc.named_scope("wmean"):
            for j in range(4):
                wt = big.tile([128, D_IN], dt.float32, tag="big", name=f"ws{j}")
                nc.scalar.dma_start(wt[:], ws[j * 128:(j + 1) * 128, :])
                nc.vector.tensor_reduce(
                    w32[:],
                    wt[:].rearrange("p (a b) -> p a b", b=128),
                    axis=AX.X, op=AL.add, apply_absolute_value=True,
                )
                nc.vector.tensor_reduce(
                    wp[:, j:j + 1], w32[:], axis=AX.X, op=AL.add
                )

        # ---- x-quant: own tokens -> resident qT tiles (vector only) ----
        HD = D_IN // 2
        HK = N_K // 2
        qT_tiles = []

        def xquant_tile(t):
            qT_t = qT_pool.tile(
                [128, N_K, 128], dt.bfloat16, tag="qT", name=f"qT{t}"
            )
            xh = []
            for h in range(2):
                xth = xq.tile([128, HD], dt.float32, tag="xq", name=f"xt{t}_{h}")
                nc.sync.dma_start(
                    xth[:], xs[t * 128:(t + 1) * 128, h * HD:(h + 1) * HD]
                )
                nc.vector.tensor_reduce(
                    (amc if h == 0 else am2)[:, t:t + 1],
                    xth[:], axis=AX.X, op=AL.max, apply_absolute_value=True,
                )
                xh.append(xth)
            # amax = max(half0, half1, EPS); s = 127/amax
            nc.vector.tensor_tensor(
                amc[:, t:t + 1], amc[:, t:t + 1], am2[:, t:t + 1], op=AL.max
            )
            nc.vector.tensor_scalar(
                amc[:, t:t + 1], amc[:, t:t + 1], EPS, None, op0=AL.max
            )
            nc.vector.reciprocal(sca[:, t:t + 1], amc[:, t:t + 1])
            nc.vector.tensor_scalar(
                sca[:, t:t + 1], sca[:, t:t + 1], 127.0, None, op0=AL.mult
            )
            for h in range(2):
                # q = rint(x*s) via fp32 magic-number round, exact in bf16
                nc.vector.tensor_scalar(
                    xh[h][:], xh[h][:], sca[:, t:t + 1], MAGIC,
                    op0=AL.mult, op1=AL.add,
                )
                qbh = qb_pool.tile(
                    [128, HD], dt.bfloat16, tag="qb", name=f"qb{t}_{h}"
                )
                nc.vector.tensor_scalar(
                    qbh[:], xh[h][:], MAGIC, None, op0=AL.subtract
                )
                nc.scalar.dma_start(
                    qT_t[:, h * HK:(h + 1) * HK, :], qbh[:], transpose=True
                )
            qT_tiles.append(qT_t)

        with nc.named_scope("xquant"), tc.tile_pool(name="xq", bufs=3) as xq:
            for t in range(4):
                xquant_tile(t)

            # ---- AllReduce of the per-partition |W| partials ----
            with nc.named_scope("wmean"):
                nc.vector.tensor_reduce(wsum[:], wp[:], axis=AX.X, op=AL.add)
                # exact partition reduce: 32x32 transpose puts the 128 values
                # on 4 rows; partition_all_reduce then adds 4 nonzeros + zeros
                nc.vector.memset(z32[:], 0.0)
                nc.vector.tensor_copy(z32[:, 0:1], wsum[:])
                nc.vector.transpose(z32t[:], z32[:])
                nc.vector.tensor_reduce(zr[:], z32t[:], axis=AX.X, op=AL.add)
                nc.gpsimd.partition_all_reduce(
                    gtot[:], zr[:], channels=128, reduce_op=bass_isa.ReduceOp.add
                )
                nc.gpsimd.dma_start(arin[:], gtot[:])
                nc.gpsimd.collective_compute(
                    "AllReduce",
                    mybir.AluOpType.add,
                    replica_groups=[list(range(N_CORES))],
                    ins=[arin.opt()],
                    outs=[arout.opt()],
                )
                nc.gpsimd.dma_start(gl[:], arout[:])
                nc.vector.tensor_scalar(
                    wme[:], gl[:], MEAN_SCALE, EPS, op0=AL.mult, op1=AL.max
                )
                nc.vector.reciprocal(swt[:], wme[:])

            for t in range(4, N_TOK_TILES):
                xquant_tile(t)

            # per-token dequant scale: amax * mean|W| / 127
            nc.vector.tensor_scalar(
                dq[:], amc[:], wme[:, 0:1], INV127, op0=AL.mult, op1=AL.mult
            )

        # ---- per-slab: ternarize+transpose one slab ahead, then matmul ----
        def stage_tern(c):
            twT_c = twTp.tile(
                [128, 4, N_K, 128], dt.bfloat16, tag="twT", name=f"twT{c}"
            )
            wts = []
            for j in range(4):
                blk = 4 * c + j
                wt = big.tile(
                    [128, D_IN], dt.float32, tag="big", name=f"wt{blk}"
                )
                nc.gpsimd.dma_start(wt[:], wf[blk * 128:(blk + 1) * 128, :])
                wts.append(wt)
            for j in range(4):
                nc.scalar.activation(
                    wts[j][:], wts[j][:], AF.Copy, scale=swt[:, 0:1]
                )
                twr = qb_pool.tile(
                    [128, D_IN], dt.bfloat16, tag="qb", name=f"twr{4 * c + j}"
                )
                nc.vector.tensor_scalar(
                    twr[:], wts[j][:], MAGIC, MAGIC, op0=AL.add, op1=AL.subtract
                )
                twc = qb_pool.tile(
                    [128, D_IN], dt.bfloat16, tag="qb", name=f"twc{4 * c + j}"
                )
                nc.vector.tensor_scalar(
                    twc[:], twr[:], 1.0, -1.0, op0=AL.min, op1=AL.max
                )
                nc.sync.dma_start(twT_c[:, j], twc[:], transpose=True)
            return twT_c

        def stage_mm(c, twT_c):
            for t in range(N_TOK_TILES):
                ps = pp.tile([128, OF_CHUNK], dt.float32, tag="ps", name=f"ps{c}_{t}")
                for k in range(N_K):
                    nc.tensor.matmul(
                        ps[:], qT_tiles[t][:, k, :], twT_c[:, :, k, :],
                        start=(k == 0), stop=(k == N_K - 1),
                    )
                ot = op.tile([128, OF_CHUNK], dt.float32, tag="ot", name=f"ot{c}_{t}")
                nc.scalar.activation(
                    ot[:], ps[:], AF.Copy, scale=dq[:, t:t + 1]
                )
                nc.gpsimd.dma_start(
                    out[t * 128:(t + 1) * 128, c * OF_CHUNK:(c + 1) * OF_CHUNK],
                    ot[:],
                )

        with nc.named_scope("mm"):
            twT_cur = stage_tern(0)
            for c in range(N_SLAB):
                twT_next = stage_tern(c + 1) if c + 1 < N_SLAB else None
                stage_mm(c, twT_cur)
                twT_cur = twT_next

    nc.compile()
    return nc


def _get_module():
    if "nc" not in _CACHE:
        _CACHE["nc"] = _build_module()
    return _CACHE["nc"]


def _make_in_maps(x2, w2):
    return [
        {
            "xs": x2[i * TOK_PC:(i + 1) * TOK_PC],
            "wf": w2,
            "ws": np.ascontiguousarray(
                w2[i * SHARD_ROWS:(i + 1) * SHARD_ROWS]
            ),
        }
        for i in range(N_CORES)
    ]


def kernel(x: np.ndarray, weight: np.ndarray) -> np.ndarray:
    from concourse.bass_utils import run_bass_kernel_spmd

    x = np.asarray(x, dtype=np.float32)
    weight = np.asarray(weight, dtype=np.float32)
    x2 = np.ascontiguousarray(x.reshape(TOK, D_IN))
    w2 = np.ascontiguousarray(weight)

    in_maps = _make_in_maps(x2, w2)
    nc = _get_module()
    res = run_bass_kernel_spmd(nc, in_maps, list(range(N_CORES)))
    out = np.concatenate([res.results[i]["out"] for i in range(N_CORES)], axis=0)
    return out.reshape(B, S, D_OUT)


# revision 4
# speedup vs baseline: 1.1328x; 1.0784x over previous
"""BitLinear (per-token int8 activation quant + ternary weight quant + matmul)
as a Bass/Tile kernel on 8 Trainium2 NeuronCores.

Strategy (data-parallel tokens + tensor-parallel weight-mean):
  - x [4,2048,4096] -> [8192,4096]; each core quantizes and matmuls its own
    1024-token slab against the FULL weight; outputs concatenate on tokens.
  - mean(|W|): each core reduces only its OWN 1/8 shard of W (extra input
    "ws", 8.4MB), then a 512B AllReduce combines the partials. This removes
    the 67MB serial full-W mean pass from the critical path.
  - q = rint(x*s) (s = 127/max(|x|) per token) and tw in {-1,0,1} are exact
    in bf16 => the bf16 matmul with fp32 PSUM accumulation is EXACT integer
    arithmetic; per-token dequant scales applied on the PSUM->SBUF copy.
  - Queue discipline: nothing that depends on the AllReduce is emitted
    before AR-independent work on any engine queue (the AR latency is long
    under this runtime and head-of-line blocks whole queues otherwise).
    sync: x loads + qT/twT transposes; scalar: shard + W-slab loads,
    x*s scale, tern scale, dequant; gpsimd: AR chain + out stores;
    vector: amax/round/clip and the mean arithmetic.
  - Ternarize slab c+1 one slab ahead of the matmul on slab c.
"""
import numpy as np
from contextlib import ExitStack

N_CORES = 8
B, S, D_IN, D_OUT = 4, 2048, 4096, 4096
TOK = B * S                  # 8192
TOK_PC = TOK // N_CORES      # 1024 tokens per core
N_TOK_TILES = TOK_PC // 128  # 8
N_K = D_IN // 128            # 32 contraction tiles
OF_CHUNK = 512
N_SLAB = D_OUT // OF_CHUNK   # 8
SHARD_ROWS = D_OUT // N_CORES  # 512 weight rows per core for the mean
EPS = 1e-5
MAGIC = float(np.float32(1.5 * 2 ** 23))   # fp32 round-to-nearest-even trick
MEAN_SCALE = float(np.float32(1.0 / (D_IN * D_OUT)))  # 2^-24, exact
INV127 = float(np.float32(1.0 / 127.0))

_CACHE = {}


def _build_module():
    import concourse.bacc as bacc
    import concourse.tile as tile
    import concourse.mybir as mybir
    import concourse.bass_isa as bass_isa

    dt = mybir.dt
    AF = mybir.ActivationFunctionType
    AL = mybir.AluOpType
    AX = mybir.AxisListType

    nc = bacc.Bacc(
        "TRN2", target_bir_lowering=False, debug=False, num_devices=N_CORES
    )
    xs = nc.dram_tensor("xs", [TOK_PC, D_IN], dt.float32, kind="ExternalInput").ap()
    wf = nc.dram_tensor("wf", [D_OUT, D_IN], dt.float32, kind="ExternalInput").ap()
    ws = nc.dram_tensor("ws", [SHARD_ROWS, D_IN], dt.float32, kind="ExternalInput").ap()
    out = nc.dram_tensor("out", [TOK_PC, D_OUT], dt.float32, kind="ExternalOutput").ap()

    with tile.TileContext(nc) as tc, ExitStack() as ctx:
        stats = ctx.enter_context(tc.tile_pool(name="stats", bufs=1))
        qT_pool = ctx.enter_context(tc.tile_pool(name="qT", bufs=N_TOK_TILES))
        big = ctx.enter_context(tc.tile_pool(name="big", bufs=2))
        qb_pool = ctx.enter_context(tc.tile_pool(name="qbp", bufs=2))
        twTp = ctx.enter_context(tc.tile_pool(name="twT", bufs=2))
        op = ctx.enter_context(tc.tile_pool(name="op", bufs=2))
        pp = ctx.enter_context(tc.tile_pool(name="pp", bufs=6, space="PSUM"))
        dram = ctx.enter_context(tc.tile_pool(name="dram", bufs=2, space="DRAM"))

        amc = stats.tile([128, N_TOK_TILES], dt.float32, tag="amc")
        am2 = stats.tile([128, N_TOK_TILES], dt.float32, tag="am2")
        sca = stats.tile([128, N_TOK_TILES], dt.float32, tag="sca")
        dq = stats.tile([128, N_TOK_TILES], dt.float32, tag="dq")
        wme = stats.tile([128, 1], dt.float32, tag="wme")
        swt = stats.tile([128, 1], dt.float32, tag="swt")
        wp = stats.tile([128, 4], dt.float32, tag="wp")
        w32 = stats.tile([128, 32], dt.float32, tag="w32")
        z32 = stats.tile([128, 32], dt.float32, tag="z32")
        z32t = stats.tile([128, 32], dt.float32, tag="z32t")
        zr = stats.tile([128, 1], dt.float32, tag="zr")
        wsum = stats.tile([128, 1], dt.float32, tag="wsum")
        gtot = stats.tile([128, 1], dt.float32, tag="gtot")
        gl = stats.tile([128, 1], dt.float32, tag="gl")

        arin = dram.tile([128, 1], dt.float32, tag="arin")
        arout = dram.tile([128, 1], dt.float32, tag="arout")

        # ---- |W| mean shard: load + reduce + trigger AllReduce early ----
        with nc.named_scope("wmean"):
            for j in range(4):
                wt = big.tile([128, D_IN], dt.float32, tag="big", name=f"ws{j}")
                nc.scalar.dma_start(wt[:], ws[j * 128:(j + 1) * 128, :])
                nc.vector.tensor_reduce(
                    w32[:],
                    wt[:].rearrange("p (a b) -> p a b", b=128),
                    axis=AX.X, op=AL.add, apply_absolute_value=True,
                )
                nc.vector.tensor_reduce(
                    wp[:, j:j + 1], w32[:], axis=AX.X, op=AL.add
                )
            nc.vector.tensor_reduce(wsum[:], wp[:], axis=AX.X, op=AL.add)
            # exact partition reduce: 32x32 transpose puts the 128 values on
            # 4 rows; partition_all_reduce then adds 4 nonzeros + 124 zeros
            nc.vector.memset(z32[:], 0.0)
            nc.vector.tensor_copy(z32[:, 0:1], wsum[:])
            nc.vector.transpose(z32t[:], z32[:])
            nc.vector.tensor_reduce(zr[:], z32t[:], axis=AX.X, op=AL.add)
            nc.gpsimd.partition_all_reduce(
                gtot[:], zr[:], channels=128, reduce_op=bass_isa.ReduceOp.add
            )
            nc.gpsimd.dma_start(arin[:], gtot[:])
            nc.gpsimd.collective_compute(
                "AllReduce",
                mybir.AluOpType.add,
                replica_groups=[list(range(N_CORES))],
                ins=[arin.opt()],
                outs=[arout.opt()],
            )

        # ---- x-quant: own tokens -> resident qT tiles (AR-independent) ----
        HD = D_IN // 2
        qT_tiles = []
        with nc.named_scope("xquant"), tc.tile_pool(name="xq", bufs=3) as xq:
            for t in range(N_TOK_TILES):
                qT_t = qT_pool.tile(
                    [128, N_K, 128], dt.bfloat16, tag="qT", name=f"qT{t}"
                )
                qbt = qb_pool.tile(
                    [128, D_IN], dt.bfloat16, tag="qb", name=f"qb{t}"
                )
                xh = []
                for h in range(2):
                    xth = xq.tile([128, HD], dt.float32, tag="xq", name=f"xt{t}_{h}")
                    nc.sync.dma_start(
                        xth[:], xs[t * 128:(t + 1) * 128, h * HD:(h + 1) * HD]
                    )
                    nc.vector.tensor_reduce(
                        (amc if h == 0 else am2)[:, t:t + 1],
                        xth[:], axis=AX.X, op=AL.max, apply_absolute_value=True,
                    )
                    xh.append(xth)
                # amax = max(half0, half1, EPS); s = 127/amax
                nc.vector.tensor_tensor(
                    amc[:, t:t + 1], amc[:, t:t + 1], am2[:, t:t + 1], op=AL.max
                )
                nc.vector.tensor_scalar(
                    amc[:, t:t + 1], amc[:, t:t + 1], EPS, None, op0=AL.max
                )
                nc.vector.reciprocal(sca[:, t:t + 1], amc[:, t:t + 1])
                nc.vector.tensor_scalar(
                    sca[:, t:t + 1], sca[:, t:t + 1], 127.0, None, op0=AL.mult
                )
                for h in range(2):
                    # x*s on the scalar engine, magic-round on vector
                    nc.scalar.activation(
                        xh[h][:], xh[h][:], AF.Copy, scale=sca[:, t:t + 1]
                    )
                    nc.vector.tensor_scalar(
                        qbt[:, h * HD:(h + 1) * HD], xh[h][:], MAGIC, MAGIC,
                        op0=AL.add, op1=AL.subtract,
                    )
                nc.sync.dma_start(qT_t[:], qbt[:], transpose=True)
                qT_tiles.append(qT_t)

        # ---- AR-dependent epilogue of the mean ----
        with nc.named_scope("wmean"):
            nc.gpsimd.dma_start(gl[:], arout[:])
            nc.vector.tensor_scalar(
                wme[:], gl[:], MEAN_SCALE, EPS, op0=AL.mult, op1=AL.max
            )
            nc.vector.reciprocal(swt[:], wme[:])
            # per-token dequant scale: amax * mean|W| / 127
            nc.vector.tensor_scalar(
                dq[:], amc[:], wme[:, 0:1], INV127, op0=AL.mult, op1=AL.mult
            )

        # ---- per-slab: ternarize+transpose one slab ahead, then matmul ----
        def stage_tern(c):
            twT_c = twTp.tile(
                [128, N_K, OF_CHUNK], dt.bfloat16, tag="twT", name=f"twT{c}"
            )
            wts = []
            for j in range(4):
                blk = 4 * c + j
                wt = big.tile(
                    [128, D_IN], dt.float32, tag="big", name=f"wt{blk}"
                )
                nc.scalar.dma_start(wt[:], wf[blk * 128:(blk + 1) * 128, :])
                wts.append(wt)
            for j in range(4):
                nc.scalar.activation(
                    wts[j][:], wts[j][:], AF.Copy, scale=swt[:, 0:1]
                )
                twr = qb_pool.tile(
                    [128, D_IN], dt.bfloat16, tag="qb", name=f"twr{4 * c + j}"
                )
                nc.vector.tensor_scalar(
                    twr[:], wts[j][:], MAGIC, MAGIC, op0=AL.add, op1=AL.subtract
                )
                twc = qb_pool.tile(
                    [128, D_IN], dt.bfloat16, tag="qb", name=f"twc{4 * c + j}"
                )
                nc.vector.tensor_scalar(
                    twc[:], twr[:], 1.0, -1.0, op0=AL.min, op1=AL.max
                )
                nc.sync.dma_start(
                    twT_c[:, :, j * 128:(j + 1) * 128], twc[:], transpose=True
                )
            return twT_c

        def stage_mm(c, twT_c):
            for t in range(N_TOK_TILES):
                ps = pp.tile([128, OF_CHUNK], dt.float32, tag="ps", name=f"ps{c}_{t}")
                for k in range(N_K):
                    nc.tensor.matmul(
                        ps[:], qT_tiles[t][:, k, :], twT_c[:, k, :],
                        start=(k == 0), stop=(k == N_K - 1),
                    )
                ot = op.tile([128, OF_CHUNK], dt.float32, tag="ot", name=f"ot{c}_{t}")
                nc.scalar.activation(
                    ot[:], ps[:], AF.Copy, scale=dq[:, t:t + 1]
                )
                nc.gpsimd.dma_start(
                    out[t * 128:(t + 1) * 128, c * OF_CHUNK:(c + 1) * OF_CHUNK],
                    ot[:],
                )

        with nc.named_scope("mm"):
            twT_cur = stage_tern(0)
            for c in range(N_SLAB):
                twT_next = stage_tern(c + 1) if c + 1 < N_SLAB else None
                stage_mm(c, twT_cur)
                twT_cur = twT_next

    nc.compile()
    return nc


def _get_module():
    if "nc" not in _CACHE:
        _CACHE["nc"] = _build_module()
    return _CACHE["nc"]


def _make_in_maps(x2, w2):
    return [
        {
            "xs": x2[i * TOK_PC:(i + 1) * TOK_PC],
            "wf": w2,
            "ws": np.ascontiguousarray(
                w2[i * SHARD_ROWS:(i + 1) * SHARD_ROWS]
            ),
        }
        for i in range(N_CORES)
    ]


def kernel(x: np.ndarray, weight: np.ndarray) -> np.ndarray:
    from concourse.bass_utils import run_bass_kernel_spmd

    x = np.asarray(x, dtype=np.float32)
    weight = np.asarray(weight, dtype=np.float32)
    x2 = np.ascontiguousarray(x.reshape(TOK, D_IN))
    w2 = np.ascontiguousarray(weight)

    in_maps = _make_in_maps(x2, w2)
    nc = _get_module()
    res = run_bass_kernel_spmd(nc, in_maps, list(range(N_CORES)))
    out = np.concatenate([res.results[i]["out"] for i in range(N_CORES)], axis=0)
    return out.reshape(B, S, D_OUT)
